# revision 2
# baseline (speedup 1.0000x reference)
"""Depthwise symmetric 7x7 Conv2d (all channels share one kernel) on 8 trn2 cores.

v2 strategy (vs baseline): same banded-matmul formulation — per output tile,
7 accumulating TensorE matmuls (H-conv via a 7-diagonal stationary matrix in
the contraction dim; W-taps via sliding the moving operand's free-dim window)
— but:
  - bf16 everywhere off-chip (x, B, y): halves HBM traffic; error ~1.7e-3,
    well under the 2e-2 gate. PSUM accumulation stays fp32.
  - Host-side plane-pair-interleaved layout [pair, row, col, 2] so every DMA
    descriptor moves 896B contiguous (>=512B avoids the 2x small-descriptor
    penalty) and so clipped matmul column ranges stay contiguous ([n, 2]
    merges to [2n]).
  - No W padding / no memsets: edge taps use clipped column ranges. dx=3
    (full width) runs first with start=True so every PSUM element's first
    write zeroes it.
  - One 2-bank PSUM tile per plane-pair ([112, 2, 256, 2] fp32; the htile
    slice starts at byte 2048 = bank 1) and ONE Activation copy per pair
    (896 free elems) instead of 2 copies of 448: the ~370ns per-instruction
    access-latency overhead made small copies nearly a co-bottleneck.
"""

import numpy as np
import ml_dtypes

import concourse.bacc as bacc
import concourse.bass as bass
import concourse.mybir as mybir
from concourse import tile
from concourse.bass_utils import run_bass_kernel_spmd

KS = 7          # kernel size
PAD = 3         # same padding
H = W = 224
N_BATCH = 16
CN = 128
N_CORES = 8
N_PLANES = N_BATCH * CN                  # 2048
N_PAIRS = N_PLANES // 2                  # 1024
PAIRS_PER_CORE = N_PAIRS // N_CORES      # 128
PLANES_PER_CORE = N_PLANES // N_CORES    # 256 (kept for test.py compat)
MT = 112        # output rows per H-tile (2 tiles cover 224)
KT = MT + PAD   # 115 input rows per H-tile (halo clipped at image edges)
WP = 256        # PSUM w stride so the htile slice lands on a bank boundary

MAXNUM = (KS * KS + KS % 2) // 2  # 25

F32 = mybir.dt.float32
BF16 = mybir.dt.bfloat16
NP_BF16 = ml_dtypes.bfloat16

# Tap order: dx=3 covers the full output width, so it goes first with
# start=True (zeroing all PSUM columns); edge taps accumulate after.
DX_ORDER = [3, 0, 1, 2, 4, 5, 6]


def _sym_weight(kv: np.ndarray) -> np.ndarray:
    """Reproduce the reference's 180-deg symmetric 7x7 kernel assembly."""
    flat = np.zeros(KS * KS, np.float32)
    idx = np.arange(MAXNUM)
    flat[idx] = kv
    flat[KS * KS - 1 - idx] = kv
    return flat.reshape(KS, KS)


def _banded_packed(k2d: np.ndarray) -> np.ndarray:
    """Banded H-conv matrices, packed [KT, 2(tile variant), 7(dx), MT].

    Variant 0 (top H-tile): input rows 0..115, output rows 0..112
        B[p, m] = k2d[p - m + 3, dx]  (band clipped at the top edge)
    Variant 1 (bottom H-tile): input rows 109..224, output rows 112..224
        B[p, m] = k2d[p - m, dx]      (band clipped at the bottom edge)
    """
    p = np.arange(KT)[:, None]
    m = np.arange(MT)[None, :]
    out = np.zeros((KT, 2, KS, MT), np.float32)
    for var, off in ((0, 3), (1, 0)):
        dy = p - m + off
        valid = (dy >= 0) & (dy < KS)
        dyc = np.clip(dy, 0, KS - 1)
        for dx in range(KS):
            out[:, var, dx, :] = np.where(valid, k2d[dyc, dx], 0.0)
    return np.ascontiguousarray(out)


def _build_nc(pairs_per_core: int) -> bass.Bass:
    nc = bacc.Bacc(
        "TRN2", target_bir_lowering=False, debug=False, num_devices=N_CORES
    )
    # x: [pair, row, col, plane-in-pair] so each DMA partition line is a
    # contiguous 896B run on both the DRAM and SBUF side, and clipped matmul
    # column windows stay contiguous.
    x = nc.dram_tensor("x", [pairs_per_core, H, W, 2], BF16, kind="ExternalInput")
    b = nc.dram_tensor("b", [KT, 2, KS, MT], BF16, kind="ExternalInput")
    # y: [pair, htile, out-row-in-tile, col, plane-in-pair]
    y = nc.dram_tensor("y", [pairs_per_core, 2, MT, W, 2], BF16, kind="ExternalOutput")

    with tile.TileContext(nc) as tc:
        with (
            tc.tile_pool(name="bpool", bufs=1) as bpool,
            tc.tile_pool(name="xpool", bufs=8) as xpool,
            tc.tile_pool(name="ppool", bufs=4, space="PSUM") as ppool,
            tc.tile_pool(name="ypool", bufs=4) as ypool,
        ):
            bsb = bpool.tile([KT, 2, KS, MT], BF16)
            nc.sync.dma_start(bsb[:], b[:])

            for g in range(pairs_per_core):
                # [part=out-row, htile, col(padded to 256), plane]
                pt = ppool.tile([MT, 2, WP, 2], F32, tag="pt")
                for t in range(2):
                    r0 = 0 if t == 0 else H - KT
                    xt = xpool.tile([KT, W, 2], BF16, tag="xt")
                    nc.sync.dma_start(xt[:], x[g, r0 : r0 + KT])
                    for i, dx in enumerate(DX_ORDER):
                        d = dx - PAD
                        a_in = max(0, d)
                        a_out = max(0, -d)
                        n = W - abs(d)
                        nc.tensor.matmul(
                            pt[:, t, a_out : a_out + n, :],
                            bsb[:, t, dx, :],
                            xt[:, a_in : a_in + n, :],
                            start=(i == 0),
                            stop=(i == KS - 1),
                        )
                yt = ypool.tile([MT, 2, W, 2], BF16, tag="yt")
                nc.scalar.copy(yt[:], pt[:, :, 0:W, :])
                nc.sync.dma_start(y[g].transpose([1, 0, 2, 3]), yt[:])
    nc.compile()
    return nc


_NC_CACHE: dict[int, bass.Bass] = {}


def _get_nc(pairs_per_core: int) -> bass.Bass:
    if pairs_per_core not in _NC_CACHE:
        _NC_CACHE[pairs_per_core] = _build_nc(pairs_per_core)
    return _NC_CACHE[pairs_per_core]


def _run(x_planes: np.ndarray, kv: np.ndarray, **spmd_kwargs):
    """x_planes: [n_planes, 224, 224] fp32; returns (out_planes, results)."""
    n_planes = x_planes.shape[0]
    n_pairs = n_planes // 2
    per_core = n_pairs // N_CORES
    assert per_core * N_CORES == n_pairs and n_pairs * 2 == n_planes
    k2d = _sym_weight(np.asarray(kv, np.float32))
    bnp = _banded_packed(k2d).astype(NP_BF16)
    # [pair, row, col, plane-in-pair] bf16
    xr = np.ascontiguousarray(
        x_planes.reshape(n_pairs, 2, H, W).transpose(0, 2, 3, 1).astype(NP_BF16)
    )
    nc = _get_nc(per_core)
    in_maps = [
        {"x": xr[i * per_core : (i + 1) * per_core], "b": bnp}
        for i in range(N_CORES)
    ]
    res = run_bass_kernel_spmd(nc, in_maps, core_ids=list(range(N_CORES)), **spmd_kwargs)
    # y device layout: [pair, htile, row, col, plane]
    yr = np.concatenate([r["y"] for r in res.results], axis=0)
    out = (
        yr.reshape(n_pairs, H, W, 2).transpose(0, 3, 1, 2).astype(np.float32)
    )
    return out.reshape(n_planes, H, W), res


def kernel(x: np.ndarray, kv: np.ndarray) -> np.ndarray:
    x = np.asarray(x, np.float32)
    planes = x.reshape(N_PLANES, H, W)
    out, _ = _run(planes, kv)
    return out.reshape(N_BATCH, CN, H, W)


# revision 3
# speedup vs baseline: 1.0151x; 1.0151x over previous
"""Depthwise symmetric 7x7 Conv2d (all channels share one kernel) on 8 trn2 cores.

v2 strategy (vs baseline): same banded-matmul formulation — per output tile,
7 accumulating TensorE matmuls (H-conv via a 7-diagonal stationary matrix in
the contraction dim; W-taps via sliding the moving operand's free-dim window)
— but:
  - bf16 everywhere off-chip (x, B, y): halves HBM traffic; error ~1.7e-3,
    well under the 2e-2 gate. PSUM accumulation stays fp32.
  - Host-side plane-pair-interleaved layout [pair, row, col, 2] so every DMA
    descriptor moves 896B contiguous (>=512B avoids the 2x small-descriptor
    penalty) and so clipped matmul column ranges stay contiguous ([n, 2]
    merges to [2n]).
  - No W padding / no memsets: edge taps use clipped column ranges. dx=3
    (full width) runs first with start=True so every PSUM element's first
    write zeroes it.
  - One 2-bank PSUM tile per plane-pair ([112, 2, 256, 2] fp32; the htile
    slice starts at byte 2048 = bank 1) and ONE Activation copy per pair
    (896 free elems) instead of 2 copies of 448: the ~370ns per-instruction
    access-latency overhead made small copies nearly a co-bottleneck.
"""

import numpy as np
import ml_dtypes

import concourse.bacc as bacc
import concourse.bass as bass
import concourse.mybir as mybir
from concourse import tile
from concourse.bass_utils import run_bass_kernel_spmd

KS = 7          # kernel size
PAD = 3         # same padding
H = W = 224
N_BATCH = 16
CN = 128
N_CORES = 8
N_PLANES = N_BATCH * CN                  # 2048
N_PAIRS = N_PLANES // 2                  # 1024
PAIRS_PER_CORE = N_PAIRS // N_CORES      # 128
PLANES_PER_CORE = N_PLANES // N_CORES    # 256 (kept for test.py compat)
MT = 112        # output rows per H-tile (2 tiles cover 224)
KT = MT + PAD   # 115 input rows per H-tile (halo clipped at image edges)
WP = 256        # PSUM w stride so the htile slice lands on a bank boundary

MAXNUM = (KS * KS + KS % 2) // 2  # 25

F32 = mybir.dt.float32
BF16 = mybir.dt.bfloat16
NP_BF16 = ml_dtypes.bfloat16

# Tap order: dx=3 covers the full output width, so it goes first with
# start=True (zeroing all PSUM columns); edge taps accumulate after.
DX_ORDER = [3, 0, 1, 2, 4, 5, 6]


def _sym_weight(kv: np.ndarray) -> np.ndarray:
    """Reproduce the reference's 180-deg symmetric 7x7 kernel assembly."""
    flat = np.zeros(KS * KS, np.float32)
    idx = np.arange(MAXNUM)
    flat[idx] = kv
    flat[KS * KS - 1 - idx] = kv
    return flat.reshape(KS, KS)


def _banded_packed(k2d: np.ndarray) -> np.ndarray:
    """Banded H-conv matrices, packed [KT, 2(tile variant), 7(dx), MT].

    Variant 0 (top H-tile): input rows 0..115, output rows 0..112
        B[p, m] = k2d[p - m + 3, dx]  (band clipped at the top edge)
    Variant 1 (bottom H-tile): input rows 109..224, output rows 112..224
        B[p, m] = k2d[p - m, dx]      (band clipped at the bottom edge)
    """
    p = np.arange(KT)[:, None]
    m = np.arange(MT)[None, :]
    out = np.zeros((KT, 2, KS, MT), np.float32)
    for var, off in ((0, 3), (1, 0)):
        dy = p - m + off
        valid = (dy >= 0) & (dy < KS)
        dyc = np.clip(dy, 0, KS - 1)
        for dx in range(KS):
            out[:, var, dx, :] = np.where(valid, k2d[dyc, dx], 0.0)
    return np.ascontiguousarray(out)


def _build_nc(pairs_per_core: int) -> bass.Bass:
    nc = bacc.Bacc(
        "TRN2", target_bir_lowering=False, debug=False, num_devices=N_CORES
    )
    # x: [pair, row, col, plane-in-pair] so each DMA partition line is a
    # contiguous 896B run on both the DRAM and SBUF side, and clipped matmul
    # column windows stay contiguous.
    x = nc.dram_tensor("x", [pairs_per_core, H, W, 2], BF16, kind="ExternalInput")
    b = nc.dram_tensor("b", [KT, 2, KS, MT], BF16, kind="ExternalInput")
    # y: [pair, htile, out-row-in-tile, col, plane-in-pair]
    y = nc.dram_tensor("y", [pairs_per_core, 2, MT, W, 2], BF16, kind="ExternalOutput")

    n_warmup = 7

    with tile.TileContext(nc) as tc:
        with (
            tc.tile_pool(name="bpool", bufs=1) as bpool,
            tc.tile_pool(name="wpool", bufs=1) as wpool,
            tc.tile_pool(name="xpool", bufs=8) as xpool,
            tc.tile_pool(name="ppool", bufs=3, space="PSUM") as ppool,
            tc.tile_pool(name="wppool", bufs=1, space="PSUM") as wppool,
            tc.tile_pool(name="ypool", bufs=4) as ypool,
        ):
            # PE warmup: dummy matmuls cover the ~4us window while the first
            # DMAs land AND ramp the Tensor engine p-state to full clock
            # before the first real matmul (saves the 3us half-rate ramp).
            wmov = wpool.tile([128, 512], BF16)
            nc.gpsimd.memset(wmov[:], 0.0)
            wpt = wppool.tile([MT, 512], F32)
            for _ in range(n_warmup):
                nc.tensor.matmul(
                    wpt[:], wmov[:, 0:MT], wmov[:], start=True, stop=True
                )

            bsb = bpool.tile([KT, 2, KS, MT], BF16)
            nc.sync.dma_start(bsb[:], b[:])

            for g in range(pairs_per_core):
                last = g == pairs_per_core - 1
                # [part=out-row, htile, col(padded to 256), plane]
                pt = ppool.tile([MT, 2, WP, 2], F32, tag="pt")
                yt = ypool.tile([MT, 2, W, 2], BF16, tag="yt")
                # The dripped last pair splits its two h-tiles across two
                # independent PSUM tiles (the second reuses the warmup bank)
                # so the t=0 copy doesn't serialize against t=1's matmuls.
                pts = [pt[:, 0], pt[:, 1]]
                if last:
                    ptb = wppool.tile([MT, WP, 2], F32)
                    pts[1] = ptb[:]
                # Input loads stay on the SP queue; output DMAs are issued
                # from the Activation queue so a pending output never blocks
                # the input-prefetch stream on SP.
                xts = []
                for t in range(2):
                    r0 = 0 if t == 0 else H - KT
                    xt = xpool.tile([KT, W, 2], BF16, tag="xt")
                    nc.sync.dma_start(xt[:], x[g, r0 : r0 + KT])
                    xts.append(xt)
                for t in range(2):
                    for i, dx in enumerate(DX_ORDER):
                        d = dx - PAD
                        a_in = max(0, d)
                        a_out = max(0, -d)
                        n = W - abs(d)
                        nc.tensor.matmul(
                            pts[t][:, a_out : a_out + n, :],
                            bsb[:, t, dx, :],
                            xts[t][:, a_in : a_in + n, :],
                            start=(i == 0),
                            stop=(i == KS - 1),
                        )
                    if last:
                        # Tail latency: drip the final pair out per h-tile so
                        # the copy of tile 0 overlaps tile 1's matmuls and the
                        # last chain is half as deep.
                        nc.scalar.copy(yt[:, t], pts[t][:, 0:W, :])
                        nc.sync.dma_start(y[g, t], yt[:, t])
                if not last:
                    nc.scalar.copy(yt[:], pt[:, :, 0:W, :])
                    nc.scalar.dma_start(y[g].transpose([1, 0, 2, 3]), yt[:])
    nc.compile()
    return nc


_NC_CACHE: dict[int, bass.Bass] = {}


def _get_nc(pairs_per_core: int) -> bass.Bass:
    if pairs_per_core not in _NC_CACHE:
        _NC_CACHE[pairs_per_core] = _build_nc(pairs_per_core)
    return _NC_CACHE[pairs_per_core]


def _run(x_planes: np.ndarray, kv: np.ndarray, **spmd_kwargs):
    """x_planes: [n_planes, 224, 224] fp32; returns (out_planes, results)."""
    n_planes = x_planes.shape[0]
    n_pairs = n_planes // 2
    per_core = n_pairs // N_CORES
    assert per_core * N_CORES == n_pairs and n_pairs * 2 == n_planes
    k2d = _sym_weight(np.asarray(kv, np.float32))
    bnp = _banded_packed(k2d).astype(NP_BF16)
    # [pair, row, col, plane-in-pair] bf16
    xr = np.ascontiguousarray(
        x_planes.reshape(n_pairs, 2, H, W).transpose(0, 2, 3, 1).astype(NP_BF16)
    )
    nc = _get_nc(per_core)
    in_maps = [
        {"x": xr[i * per_core : (i + 1) * per_core], "b": bnp}
        for i in range(N_CORES)
    ]
    res = run_bass_kernel_spmd(nc, in_maps, core_ids=list(range(N_CORES)), **spmd_kwargs)
    # y device layout: [pair, htile, row, col, plane]
    yr = np.concatenate([r["y"] for r in res.results], axis=0)
    out = (
        yr.reshape(n_pairs, H, W, 2).transpose(0, 3, 1, 2).astype(np.float32)
    )
    return out.reshape(n_planes, H, W), res


def kernel(x: np.ndarray, kv: np.ndarray) -> np.ndarray:
    x = np.asarray(x, np.float32)
    planes = x.reshape(N_PLANES, H, W)
    out, _ = _run(planes, kv)
    return out.reshape(N_BATCH, CN, H, W)
